# revision 28
# baseline (speedup 1.0000x reference)
"""MoE (8 experts, top-2) Trainium2 kernel.

Strategy: expert-parallel with two-segment load balancing across the 8
NeuronCores. The gate (~0.03% of FLOPs) runs on the host and produces
routing metadata only. Device compute is the routed expert MLP:

    y_e = relu(x_e @ W1[e] + b1[e]) @ W2[e] + b2[e]

Every core runs the SAME program (SPMD) processing C = c1 + c2 token
columns in two fixed-size segments; each segment has its own full
weight set passed as input. Expert token lists are bin-packed into the
8 c1-bins and 8 c2-bins (an expert may span several bins on several
cores), so C ~ max(1024, sum/8) instead of max_e(n_e). The host picks
(c1, c2) per input counts (compile cached per signature).

Per-core device layout (transposed activations; biases per-partition):
  MM1:  hT[FF, C] = W1.T @ xT   (+b1, relu)   per-segment weights
  MM2:  yT[H, C]  = W2.T @ hT   (+b2)         per-segment weights
fp16 matmuls (1 cycle/col @2.4GHz), fp32 PSUM accumulation, fp16 y out.

Cold-start plan (the window is HBM-bandwidth saturated):
- 14 dummy matmuls on a memset scratch tile keep the PE busy from the
  end of the NEFF preamble, completing the DVFS ramp (0.65->2.4GHz
  takes ~3us of continuous execution) while the first inputs land.
- x k-tiles stream on the SP ring (kh=0 first); w1 for kf0/kf1 lands
  as quarter tiles on the Activation ring; kf0/kf1 run kh-MAJOR with
  all 6 PSUM groups open so the PE only ever waits on the x k-tile
  currently landing. b1/b2 ride the GpSimd SWDGE.
- w2 loads interleave into the w1 stream at kf=4/7/10 instead of
  polluting the cold window (they are not needed until MM2, ~75us).
"""

import numpy as np

# ---------------------------------------------------------------- config
NUM_EXPERTS = 8
TOP_K = 2
B, S, H = 4, 1024, 1024
FF = 2 * H
T = B * S
P = 128
KH = H // P    # 8 k-tiles over H
KF = FF // P   # 16 k-tiles over FF
NTILE = 512    # max matmul moving free dim (one PSUM bank of fp32)
MM_DT_NAME = "f16"  # one of: bf16, f16, f32r, f32

PROFILE = False       # set True (from test.py) to trace + record HW time
LAST_EXEC_NS = None
LAST_RESULTS = None

_cache = {}


def _mm_dt():
    import concourse.mybir as mybir
    import ml_dtypes

    return {
        "bf16": (mybir.dt.bfloat16, ml_dtypes.bfloat16),
        "f16": (mybir.dt.float16, np.float16),
        "f32r": (mybir.dt.float32r, np.float32),
        "f32": (mybir.dt.float32, np.float32),
    }[MM_DT_NAME]


def _ntiles(seg0, seg_len):
    """Split a segment into <=NTILE equal-ish column tiles."""
    if seg_len == 0:
        return []
    n = -(-seg_len // NTILE)
    base = seg_len // n
    rem = seg_len - base * n
    out = []
    o = seg0
    for j in range(n):
        sz = base + (1 if j < rem else 0)
        out.append((o, sz))
        o += sz
    return out


def _build(c1, c2):
    """Per-core Bass program: two expert segments of c1 and c2 columns."""
    import concourse.bass as bass
    import concourse.mybir as mybir
    import concourse.tile as tile
    from concourse import bacc

    mm_dt, _ = _mm_dt()
    f32 = mybir.dt.float32
    ts, ds = bass.ts, bass.ds

    C = c1 + c2
    segs = [(0, c1, 0), (c1, c2, 1)]  # (col0, len, weight-set index)
    tiles_by_seg = [_ntiles(s0, sl) for (s0, sl, _) in segs]

    nc = bacc.Bacc("TRN2", debug=False, num_devices=NUM_EXPERTS)

    xt_d = nc.dram_tensor("xt_d", [KH, P, C], mm_dt, kind="ExternalInput")
    # weight set A in cols [0 : KH*P / KF*P], set B after it
    w1_d = nc.dram_tensor("w1_d", [KF, P, 2 * KH * P], mm_dt, kind="ExternalInput")
    b1_d = nc.dram_tensor("b1_d", [P, 2 * KF], f32, kind="ExternalInput")
    w2_d = nc.dram_tensor("w2_d", [KH, P, 2 * KF * P], mm_dt, kind="ExternalInput")
    b2_d = nc.dram_tensor("b2_d", [P, 2 * KH], f32, kind="ExternalInput")
    y_d = nc.dram_tensor("y_d", [KH, P, C], mm_dt, kind="ExternalOutput")

    with tile.TileContext(nc) as tc:
        with (
            tc.tile_pool(name="const", bufs=1) as const,
            tc.tile_pool(name="xtp", bufs=1) as xtp,
            tc.tile_pool(name="hp", bufs=1) as hp,
            tc.tile_pool(name="w1q", bufs=1) as w1q,   # kf0/kf1 quarter tiles
            tc.tile_pool(name="w1f", bufs=5) as w1f,   # full [P, 2*KH*P] tiles
            tc.tile_pool(name="w2p", bufs=3) as w2p,
            tc.tile_pool(name="yp", bufs=3) as yp,
            tc.tile_pool(name="pheld", bufs=1, space="PSUM") as pheld,
            tc.tile_pool(name="prot", bufs=2, space="PSUM") as prot,
        ):
            # PE warmup: dummy matmuls on a memset scratch tile ramp the
            # tensor engine to full DVFS p-state while input DMAs land.
            warm = const.tile([P, NTILE], mm_dt)
            nc.gpsimd.memset(warm[:], 0)

            # x k-tiles first on the SP ring: the first matmul's moving
            # operand; kh=0 lands first.
            xts = []
            for kh in range(KH):
                xk = xtp.tile([P, C], mm_dt, tag=f"xt{kh}")
                nc.sync.dma_start(xk[:], xt_d.ap()[kh])
                xts.append(xk)

            # biases via GpSimd SWDGE: off both HW rings, needed only at
            # the first MM1 eviction (~7us after ring start).
            b1t = const.tile([P, 2 * KF], f32)
            nc.gpsimd.dma_start(b1t[:], b1_d.ap())
            b2t = const.tile([P, 2 * KH], f32)
            nc.gpsimd.dma_start(b2t[:], b2_d.ap())

            h = hp.tile([P, KF, C], mm_dt)

            wacc = prot.tile([P, NTILE], f32, tag="acc", name="wacc")
            for _ in range(14):
                nc.tensor.matmul(
                    wacc[:, :NTILE],
                    warm[:, 0:P],
                    warm[:],
                    start=True,
                    stop=True,
                )

            # w2 loads ride the Activation ring interleaved into the w1
            # stream (kf=4/7/10) so they stay out of the bandwidth-
            # saturated cold window where x races the PE; m>=3 issue as
            # w2p bufs free during MM1 tail / MM2.
            w2_tiles = {}

            def load_w2(m):
                t = w2p.tile([P, 2 * KF * P], mm_dt, tag="w2f")
                nc.scalar.dma_start(t[:], w2_d.ap()[m])
                w2_tiles[m] = t

            # ---- MM1: hT[kf] = relu(W1.T @ xT + b1), per segment ----
            # kf0/kf1 are streamed kh-major (all their PSUM groups open at
            # once) so the PE only ever waits on the x k-tile that is
            # currently landing; their weights arrive as quarter tiles in
            # matching order. kf>=2 run kf-major with one 1MB DMA each.
            SPLIT_KFS = (0, 1)
            w1_parts = {kf: [None] * 4 for kf in SPLIT_KFS}

            def load_w1_quarter(kf, q):
                t = w1q.tile([P, 512], mm_dt, tag=f"w1q{kf}_{q}")
                nc.scalar.dma_start(t[:], w1_d.ap()[kf, :, q * 512 : (q + 1) * 512])
                w1_parts[kf][q] = t

            # quarters q: 0 = setA kh0-3, 1 = setA kh4-7, 2 = setB kh0-3,
            # 3 = setB kh4-7; land low-kh halves of both sets first
            for kf in SPLIT_KFS:
                load_w1_quarter(kf, 0)
                load_w1_quarter(kf, 2)
            for kf in SPLIT_KFS:
                load_w1_quarter(kf, 1)
                load_w1_quarter(kf, 3)

            seg_tiles = []  # (si, widx, n0, nsz)
            for si, (s0, slen, widx) in enumerate(segs):
                if slen == 0:
                    continue
                for (n0, nsz) in tiles_by_seg[si]:
                    seg_tiles.append((si, widx, n0, nsz))

            held = {}
            for kh in range(KH):
                for kf in SPLIT_KFS:
                    for ti, (si, widx, n0, nsz) in enumerate(seg_tiles):
                        key = (kf, ti)
                        if kh == 0:
                            held[key] = pheld.tile(
                                [P, NTILE],
                                f32,
                                tag=f"acc{kf}_{ti}",
                                name=f"acc{kf}_{ti}",
                                bufs=1,
                            )
                        acc = held[key]
                        col = widx * KH * P + kh * P
                        wt = w1_parts[kf][col // 512]
                        nc.tensor.matmul(
                            acc[:, :nsz],
                            wt[:, ds(col % 512, P)],
                            xts[kh][:, ds(n0, nsz)],
                            start=(kh == 0),
                            stop=(kh == KH - 1),
                            skip_group_check=True,
                        )
            for kf in SPLIT_KFS:
                for ti, (si, widx, n0, nsz) in enumerate(seg_tiles):
                    nc.scalar.activation(
                        h[:, kf, ds(n0, nsz)],
                        held[(kf, ti)][:, :nsz],
                        mybir.ActivationFunctionType.Relu,
                        bias=b1t[:, widx * KF + kf : widx * KF + kf + 1],
                    )

            for kf in range(len(SPLIT_KFS), KF):
                w1t = w1f.tile([P, 2 * KH * P], mm_dt, tag="w1f")
                nc.scalar.dma_start(w1t[:], w1_d.ap()[kf])
                if kf in (4, 7, 10):
                    load_w2(len(w2_tiles))
                for (si, widx, n0, nsz) in seg_tiles:
                    acc = prot.tile([P, NTILE], f32)
                    for kh in range(KH):
                        nc.tensor.matmul(
                            acc[:, :nsz],
                            w1t[:, ds(widx * KH * P + kh * P, P)],
                            xts[kh][:, ds(n0, nsz)],
                            start=(kh == 0),
                            stop=(kh == KH - 1),
                        )
                    nc.scalar.activation(
                        h[:, kf, ds(n0, nsz)],
                        acc[:, :nsz],
                        mybir.ActivationFunctionType.Relu,
                        bias=b1t[:, widx * KF + kf : widx * KF + kf + 1],
                    )

            # ---- MM2: yT[m] = W2.T @ hT + b2, per segment ----
            for m in range(KH):
                if m not in w2_tiles:
                    load_w2(m)
                if m + 1 < KH and (m + 1) not in w2_tiles:
                    load_w2(m + 1)
                w2t = w2_tiles[m]
                # last m emits its widest tile first so the final
                # activation + y DMA tail is short
                tiles_m = seg_tiles
                if m == KH - 1:
                    tiles_m = sorted(seg_tiles, key=lambda t: -t[3])
                for (si, widx, n0, nsz) in tiles_m:
                    acc = prot.tile([P, NTILE], f32)
                    for k in range(KF):
                        nc.tensor.matmul(
                            acc[:, :nsz],
                            w2t[:, ds(widx * KF * P + k * P, P)],
                            h[:, k, ds(n0, nsz)],
                            start=(k == 0),
                            stop=(k == KF - 1),
                        )
                    yt = yp.tile([P, NTILE], mm_dt)
                    nc.scalar.activation(
                        yt[:, :nsz],
                        acc[:, :nsz],
                        mybir.ActivationFunctionType.Identity,
                        bias=b2t[:, widx * KH + m : widx * KH + m + 1],
                    )
                    nc.sync.dma_start(y_d.ap()[m, :, ds(n0, nsz)], yt[:, :nsz])

    nc.compile()
    return nc


def _install_profile_shim():
    """Make run_bass_kernel_spmd(trace=True) work under axon in this
    container (the boot-time antenv.axon_hooks install is absent)."""
    import contextlib
    import ctypes
    import sys
    import types

    if "antenv.axon_hooks" in sys.modules:
        return
    so_path = "/opt/axon/libaxon_pjrt.so"
    lib = ctypes.CDLL(so_path)
    if not hasattr(lib, "axon_start_nrt_profile"):
        return
    lib.axon_start_nrt_profile.argtypes = [
        ctypes.POINTER(ctypes.c_int64),
        ctypes.c_size_t,
    ]
    lib.axon_start_nrt_profile.restype = ctypes.c_int64
    lib.axon_stop_nrt_profile.argtypes = [ctypes.c_char_p]
    lib.axon_stop_nrt_profile.restype = ctypes.c_int64

    @contextlib.contextmanager
    def _hook(output_dir, device_ids):
        import jax

        jax.devices()
        if device_ids:
            ids = (ctypes.c_int64 * len(device_ids))(*device_ids)
            rc = lib.axon_start_nrt_profile(ids, len(device_ids))
        else:
            rc = lib.axon_start_nrt_profile(None, 0)
        if rc != 0:
            raise RuntimeError(f"axon_start_nrt_profile rc={rc}")
        try:
            yield
        finally:
            n = lib.axon_stop_nrt_profile(str(output_dir).encode())
            print(f"ntff profile: {n} file(s) in {output_dir}", file=sys.stderr)

    mod = types.ModuleType("antenv.axon_hooks")
    mod.get_axon_ntff_profile_hook = lambda: _hook
    mod.set_axon_ntff_profile_hook = lambda h: None
    sys.modules["antenv.axon_hooks"] = mod

    import concourse.bass_utils as bu

    bu.upload_artifacts = lambda tmpdir: str(tmpdir)


# ---------------------------------------------------------------- host side

def _route(xf, Wg, bg):
    """Top-2 routing on host, float64 scoring. Returns (top2 [T,2] int,
    w [T,2] float32 renormalized combine weights)."""
    logits = xf.astype(np.float64) @ Wg.astype(np.float64) + bg.astype(np.float64)
    top2 = np.argsort(-logits, axis=-1, kind="stable")[:, :TOP_K]
    lv = np.take_along_axis(logits, top2, axis=1)
    lv = lv - lv.max(axis=1, keepdims=True)
    ev = np.exp(lv)
    w = ev / ev.sum(axis=1, keepdims=True)
    return top2, w.astype(np.float32)


def _solve_bins(counts, align=2, min_seg=256):
    """Pick (c1, c2) minimizing C = c1+c2 such that expert token counts
    fit into 8 bins of c1 plus 8 bins of c2 (each bin single-expert).
    Segments are kept >= min_seg so matmul N stays large enough to hide
    PE weight loads. Returns (c1, c2, assign), assign[e] = (a_e, b_e)."""
    counts = list(int(c) for c in counts)
    nE = len(counts)
    lo = max(
        -(-sum(counts) // 8),
        -(-max(counts) // 16) if max(counts) else 1,
        2 * min_seg,
    )
    lo = -(-lo // align) * align

    def feasible(c1, c2):
        # DP over experts: state = sum_a used, value = min sum_b
        INF = 10**9
        dp = [INF] * 9
        dp[0] = 0
        choice = []
        for n in counts:
            opts = []
            for a in range(0, 9):
                rem = n - a * c1
                b = 0 if rem <= 0 else -(-rem // c2)
                if b <= 8:
                    opts.append((a, b))
                if rem <= 0:
                    break
            ndp = [INF] * 9
            pick = [None] * 9
            for ua in range(9):
                if dp[ua] == INF:
                    continue
                for (a, b) in opts:
                    if ua + a <= 8 and dp[ua] + b < ndp[ua + a]:
                        ndp[ua + a] = dp[ua] + b
                        pick[ua + a] = (ua, a, b)
            choice.append(pick)
            dp = ndp
        best_ua, best_b = None, INF
        for ua in range(9):
            if dp[ua] < best_b:
                best_ua, best_b = ua, dp[ua]
        if best_b > 8:
            return None
        # backtrack
        assign = [None] * nE
        ua = best_ua
        # recompute backwards: need per-expert chosen (a, b)
        # choice[e][ua_after] = (ua_before, a, b) is ambiguous across
        # experts; redo forward with stored picks
        ua_states = [best_ua]
        for e in range(nE - 1, -1, -1):
            pk = choice[e][ua_states[-1]]
            if pk is None:
                return None
            ua_before, a, b = pk
            assign[e] = (a, b)
            ua_states.append(ua_before)
        return assign

    for C in range(lo, 16 * lo, align):
        # c1 from C/2 up; prefer balanced splits (smaller max segment)
        best = None
        for c1 in range(C // 2 // align * align, C - min_seg + 1, align):
            c2 = C - c1
            if c2 < min_seg:
                break
            asg = feasible(c1, c2)
            if asg is not None:
                best = (c1, c2, asg)
                break
        if best is not None:
            return best
    raise RuntimeError(f"no feasible bin packing for counts={counts}")


def _prep_weight_set(W1, b1, W2, b2, e, np_dt):
    w1g = (
        W1[e]
        .reshape(KH, P, KF, P)
        .transpose(2, 1, 0, 3)
        .reshape(KF, P, KH * P)
        .astype(np_dt)
    )
    w2g = (
        W2[e]
        .reshape(KF, P, KH, P)
        .transpose(2, 1, 0, 3)
        .reshape(KH, P, KF * P)
        .astype(np_dt)
    )
    b1g = np.ascontiguousarray(b1[e].reshape(KF, P).T).astype(np.float32)
    b2g = np.ascontiguousarray(b2[e].reshape(KH, P).T).astype(np.float32)
    return w1g, w2g, b1g, b2g


def kernel(x, Wg, bg, W1, b1, W2, b2):
    global LAST_EXEC_NS, LAST_RESULTS

    x = np.asarray(x, dtype=np.float32)
    Wg = np.asarray(Wg, dtype=np.float32)
    bg = np.asarray(bg, dtype=np.float32)
    W1 = np.asarray(W1, dtype=np.float32)
    b1 = np.asarray(b1, dtype=np.float32)
    W2 = np.asarray(W2, dtype=np.float32)
    b2 = np.asarray(b2, dtype=np.float32)

    _, np_dt = _mm_dt()
    if PROFILE:
        _install_profile_shim()

    from concourse.bass_utils import run_bass_kernel_spmd

    xf = x.reshape(T, H)
    top2, w = _route(xf, Wg, bg)

    # token lists per expert
    idx_list = []
    wgt_list = []
    for e in range(NUM_EXPERTS):
        mask = top2 == e  # [T, 2]
        idx = np.where(mask.any(axis=1))[0]
        slot = mask[idx, 1].astype(np.int64)
        idx_list.append(idx)
        wgt_list.append(w[idx, slot])

    counts = [len(i) for i in idx_list]
    c1, c2, assign = _solve_bins(counts)
    C = c1 + c2

    # lay expert pieces into bins: bins_a[8], bins_b[8] hold
    # (expert, tok_slice) or None
    bins_a, bins_b = [], []
    for e in range(NUM_EXPERTS):
        a, bcnt = assign[e]
        pos = 0
        n = counts[e]
        for _ in range(a):
            take = min(c1, n - pos)
            bins_a.append((e, pos, max(take, 0)))
            pos += take
        for _ in range(bcnt):
            take = min(c2, n - pos)
            bins_b.append((e, pos, max(take, 0)))
            pos += take
        assert pos >= n, (e, counts[e], assign[e], c1, c2)
    while len(bins_a) < 8:
        bins_a.append((0, 0, 0))
    while len(bins_b) < 8:
        bins_b.append((0, 0, 0))

    key = (c1, c2)
    if key not in _cache:
        _cache[key] = _build(c1, c2)
    nc = _cache[key]

    wcache = {}

    def wset(e):
        if e not in wcache:
            wcache[e] = _prep_weight_set(W1, b1, W2, b2, e, np_dt)
        return wcache[e]

    in_maps = []
    core_meta = []
    for core in range(NUM_EXPERTS):
        eA, posA, lenA = bins_a[core]
        eB, posB, lenB = bins_b[core]
        w1A, w2A, b1A, b2A = wset(eA)
        w1B, w2B, b1B, b2B = wset(eB)
        xe = np.zeros((H, C), dtype=np_dt)
        idxA = idx_list[eA][posA : posA + lenA]
        idxB = idx_list[eB][posB : posB + lenB]
        if lenA:
            xe[:, :lenA] = xf[idxA].T.astype(np_dt)
        if lenB:
            xe[:, c1 : c1 + lenB] = xf[idxB].T.astype(np_dt)
        in_maps.append(
            {
                "xt_d": xe.reshape(KH, P, C),
                "w1_d": np.concatenate([w1A, w1B], axis=2),
                "b1_d": np.concatenate([b1A, b1B], axis=1),
                "w2_d": np.concatenate([w2A, w2B], axis=2),
                "b2_d": np.concatenate([b2A, b2B], axis=1),
            }
        )
        core_meta.append((eA, idxA, posA, eB, idxB, posB))

    res = run_bass_kernel_spmd(
        nc,
        in_maps,
        core_ids=list(range(NUM_EXPERTS)),
        trace=bool(PROFILE),
    )
    if PROFILE:
        LAST_EXEC_NS = res.exec_time_ns
        LAST_RESULTS = res

    out = np.zeros((T, H), dtype=np.float32)
    for core in range(NUM_EXPERTS):
        eA, idxA, posA, eB, idxB, posB = core_meta[core]
        yT = res.results[core]["y_d"].reshape(H, C).astype(np.float32)
        if len(idxA):
            we = wgt_list[eA][posA : posA + len(idxA)]
            out[idxA] += we[:, None] * yT[:, : len(idxA)].T
        if len(idxB):
            we = wgt_list[eB][posB : posB + len(idxB)]
            out[idxB] += we[:, None] * yT[:, c1 : c1 + len(idxB)].T

    return out.reshape(B, S, H)
